# revision 45
# baseline (speedup 1.0000x reference)
"""Two-layer GCN (GCNConv -> ReLU -> GCNConv -> log_softmax) on 8 Trainium2
NeuronCores.

Strategy (graph/data parallel node partitioning), rev A (fp8):
  * Destination nodes are dealt round-robin by in-degree across cores and
    tiles (host-side) so per-(core,tile,chunk) edge buckets are balanced.
  * Phase 1: each core computes g = fp8(16 * dinv * (x_shard @ W1)) for its
    own nodes via fp8 DoubleRow matmuls (x in e4m3, W1*64 in e4m3, fp32 PSUM),
    stores its g-table shard as fp8 rows padded to 1024B.
  * Phase 2: AllGather the fp8 g table (103 MB full table).
  * Phase 3: per dst tile, `dma_gather` pulls 1024B source rows for all
    in-edges (edges bucketed by table quarter-chunk for int16 indices);
    a host-prebuilt fp8 selection matrix S is streamed from HBM and the
    per-destination segment-sum becomes fp8 DoubleRow PE matmuls (block
    pairs) accumulated in PSUM. Epilogue: out1 = relu(dinv/16*acc + b1);
    g2 = dinv * (out1 @ W2) via PE transposes; g2 stored fp16 in 256B rows.
  * Phase 3.5: AllGather g2 (fp8, 256B rows).
  * Phase 4: gather 256B g2 rows per edge, fp8 DoubleRow matmul against the
    same streamed S, then dinv, b2 and log_softmax.

  The global table is laid out as [all cores' tiles 0..T/2-1 | tiles
  T/2..T-1] so each AllGather splits into two collectives and phase 3/4
  chunk-0/1 work overlaps the second half's transfer.  Bucket padding uses
  negative indices (skipped by the gather ucode per-core), with at least 16
  real descriptors per call to keep the completion semaphore sane.

Self-contained: hardcodes shapes; only needs the container toolchain at
/opt/trn_rl_repo.
"""

import os
import sys

for _p in ("/opt/trn_rl_repo",):
    if _p not in sys.path:
        sys.path.insert(0, _p)

import ml_dtypes
import numpy as np

import concourse.bacc as bacc
import concourse.bass as bass
import concourse.tile as tile
from concourse import bass_utils, mybir
from concourse.masks import make_identity

P = 128
FP16 = mybir.dt.float16
FP8 = mybir.dt.float8e4
F32 = mybir.dt.float32
I16 = mybir.dt.int16
I32 = mybir.dt.int32
AX = mybir.AxisListType
ALU = mybir.AluOpType
ACT = mybir.ActivationFunctionType
DR = mybir.MatmulPerfMode.DoubleRow
NPF8 = ml_dtypes.float8_e4m3fn
SPLIT_AG = bool(int(os.environ.get("GCN_SPLITAG", "1")))
DYN_CNT = bool(int(os.environ.get("GCN_DYN", "1")))

GS = 16.0     # g-table fp8 scale: stored g_q = g * GS
WS = 64.0     # W1 fp8 scale: stored w_q = W1 * WS


class Cfg:
    def __init__(self, n_nodes=100000, n_cores=8, f_in=1433, f_mid=789, f_out=7,
                 n_chunks=4, mm_free=512):
        self.n_nodes = n_nodes
        self.n_cores = n_cores
        self.f_in = f_in
        self.kc = (f_in + P - 1) // P          # k-chunks for layer-1 matmul
        assert self.kc % 2 == 0
        self.f_mid = f_mid
        self.fmp = ((f_mid + 255) // 256) * 256   # fp8 row padded to 256B: 1024
        self.kc2 = (f_mid + P - 1) // P        # k-chunks for layer-2 matmul
        self.f_out = f_out
        self.ns = n_nodes // n_cores           # nodes per shard (pre-pad)
        assert self.ns * n_cores == n_nodes
        self.t = (self.ns + P - 1) // P        # dst tiles per core
        assert self.t % 2 == 0
        self.ta = self.t // 2                  # tiles in table half A
        self.nsp = self.t * P                  # padded shard size
        self.ntot = self.nsp * n_cores         # padded global table rows
        self.na = self.ntot // 2               # rows in table half A
        self.n_chunks = n_chunks               # int16 table chunks
        assert self.ntot % n_chunks == 0
        self.vc = self.ntot // n_chunks        # rows per chunk
        assert self.vc < 32768
        self.mm_free = mm_free
        # set by preprocess:
        self.kb = None                         # [t][cb] blocks per bucket
        self.bt = None                         # [t] total blocks per tile
        self.btmax = None
        self.kbmax = None


# ----------------------------------------------------------------- host side

def preprocess(x, edge_index, W1, b1, W2, b2, cfg):
    """Shard + permute nodes, bucket edges by (dst tile, src chunk)."""
    N, C = cfg.n_nodes, cfg.n_cores
    src = np.asarray(edge_index[0], dtype=np.int64)
    dst = np.asarray(edge_index[1], dtype=np.int64)
    loop = np.arange(N, dtype=np.int64)
    src = np.concatenate([src, loop])
    dst = np.concatenate([dst, loop])

    deg = np.bincount(dst, minlength=N).astype(np.float64)
    dinv = (1.0 / np.sqrt(deg)).astype(np.float32)

    # deal nodes round-robin by in-degree across cores, then snake across
    # tiles within each core, to balance (core, tile, chunk) bucket counts.
    indeg = np.bincount(dst, minlength=N)
    order_glob = np.argsort(-indeg, kind="stable")
    shard_of = np.zeros(N, dtype=np.int64)
    node_tile = np.zeros(N, dtype=np.int64)
    node_col = np.zeros(N, dtype=np.int64)
    pg = np.zeros(N, dtype=np.int64)
    nodes_of = []
    snake = np.concatenate([np.arange(cfg.t), np.arange(cfg.t)[::-1]])
    tiles_seq = np.tile(snake, (P + 1) // 2 + 1)[: cfg.nsp]
    for c in range(C):
        order = order_glob[c::C]               # this core's nodes, by degree
        shard_of[order] = c
        tl = tiles_seq[: cfg.ns]
        node_tile[order] = tl
        pos = np.argsort(tl, kind="stable")
        cols = np.empty(cfg.ns, dtype=np.int64)
        tile_sorted = tl[pos]
        start = np.searchsorted(tile_sorted, np.arange(cfg.t))
        cols[pos] = np.arange(cfg.ns) - start[tile_sorted]
        node_col[order] = cols
        if SPLIT_AG:
            half_b = tl >= cfg.ta
            pg[order] = np.where(
                half_b,
                cfg.na + c * cfg.ta * P + (tl - cfg.ta) * P + cols,
                c * cfg.ta * P + tl * P + cols)
        else:
            pg[order] = c * cfg.nsp + tl * P + cols
        nv = np.full(cfg.nsp, -1, dtype=np.int64)
        nv[tl * P + cols] = order
        nodes_of.append(nv)

    # ---- bucket edges by (core, dst tile, src chunk)
    e_shard = shard_of[dst]
    e_tile = node_tile[dst]
    e_src_pg = pg[src]
    e_chunk = e_src_pg // cfg.vc
    e_dcol = node_col[dst]
    NB = cfg.n_chunks
    counts = np.zeros((C, cfg.t, NB), dtype=np.int64)
    np.add.at(counts, (e_shard, e_tile, e_chunk), 1)
    kb = ((counts.max(axis=0) + P - 1) // P).astype(np.int64)   # [t, NB]
    kb = np.maximum(kb, 1)
    # shared (max-over-cores) real index count per bucket; trailing slots up
    # to kb*128 are -1 and skipped by the gather ucode
    cfg.cmax = np.maximum(counts.max(axis=0), 16).astype(np.int64)
    # per-tile block totals must be even for DoubleRow pairing
    odd = kb.sum(axis=1) % 2 == 1
    kb[odd, NB - 1] += 1
    cfg.kb = kb
    cfg.bt = kb.sum(axis=1)                   # [t]
    cfg.btmax = int(cfg.bt.max())
    if cfg.btmax % 2:
        cfg.btmax += 1
    cfg.kbmax = int(kb.max())
    nblk_tot = int(cfg.bt.sum())

    order_all = np.lexsort((e_src_pg, e_chunk, e_tile, e_shard))
    s_sorted = (e_src_pg - e_chunk * cfg.vc)[order_all].astype(np.int16)
    d_sorted = e_dcol[order_all].astype(np.int64)
    key = (e_shard * cfg.t + e_tile)[order_all] * NB + e_chunk[order_all]
    bounds = np.searchsorted(key, np.arange(C * cfg.t * NB + 1))

    # block offsets per (t, cb)
    blk_off = np.zeros((cfg.t, NB), dtype=np.int64)
    run = 0
    for t in range(cfg.t):
        for cb in range(NB):
            blk_off[t, cb] = run
            run += kb[t, cb]

    xpad = np.zeros((cfg.kc * P, N), dtype=NPF8)
    xq = np.clip(np.asarray(x, dtype=np.float32), -240, 240)
    xpad[: cfg.f_in, :] = xq.T.astype(NPF8)
    w1h = np.zeros((P, cfg.kc, cfg.f_mid), dtype=NPF8)
    w1t = np.zeros((cfg.kc * P, cfg.f_mid), dtype=np.float32)
    w1t[: cfg.f_in] = np.clip(np.asarray(W1, dtype=np.float32) * WS, -240, 240)
    w1h[:] = w1t.reshape(cfg.kc, P, cfg.f_mid).transpose(1, 0, 2).astype(NPF8)
    w2h = np.zeros((P, cfg.kc2, cfg.f_out), dtype=np.float32)
    w2t = np.zeros((cfg.kc2 * P, cfg.f_out), dtype=np.float32)
    w2t[: cfg.f_mid] = np.asarray(W2, dtype=np.float32)
    w2h[:] = w2t.reshape(cfg.kc2, P, cfg.f_out).transpose(1, 0, 2)
    b1r = np.tile(np.asarray(b1, dtype=np.float32)[None, :], (P, 1))
    b2r = np.zeros((P, 8), dtype=np.float32)
    b2r[:, : cfg.f_out] = np.asarray(b2, dtype=np.float32)[None, :]

    cols128 = np.arange(P, dtype=np.int64)
    in_maps = []
    for c in range(C):
        nv = nodes_of[c]
        valid = nv >= 0
        xs = np.zeros((cfg.kc * P, cfg.nsp), dtype=NPF8)
        xs[:, valid] = xpad[:, nv[valid]]
        xt = np.ascontiguousarray(xs.reshape(cfg.kc, P, cfg.nsp).transpose(1, 0, 2))
        dvt = np.zeros(cfg.nsp, dtype=np.float32)
        dvt[valid] = dinv[nv[valid]]
        dv = np.ascontiguousarray(dvt.reshape(cfg.t, P).T)
        # idx: per (t, cb): kb*128 int16, idx j at [j%16, off*8 + j//16]
        eidx = np.zeros((P, nblk_tot * 8), dtype=np.int16)
        # S: per block b, S[p, b, col] = 1 if edge slot (b*128+p) -> dst col
        sful = np.zeros((P, nblk_tot, P), dtype=NPF8)
        for t in range(cfg.t):
            for cb in range(NB):
                lo = bounds[(c * cfg.t + t) * NB + cb]
                hi = bounds[(c * cfg.t + t) * NB + cb + 1]
                cnt = hi - lo
                nsl = int(kb[t, cb]) * P
                off = int(blk_off[t, cb])
                # trailing -1 idxs are skipped by the gather ucode; keep at
                # least 16 non-negative so every SDMA engine gets a desc
                cmv = int(cfg.cmax[t, cb])
                ai = np.full(nsl, -1 if DYN_CNT else 0, dtype=np.int16)
                if DYN_CNT:
                    ai[:cmv] = 0
                ai[:cnt] = s_sorted[lo:hi]
                eidx[:, off * 8: off * 8 + nsl // 16] = np.tile(
                    ai.reshape(nsl // 16, 16).T, (8, 1))
                ad = np.full(nsl, -1, dtype=np.int64)
                ad[:cnt] = d_sorted[lo:hi]
                blkd = ad.reshape(int(kb[t, cb]), P)      # [kb, 128] dst cols
                sful[:, off: off + int(kb[t, cb]), :] = (
                    blkd.T[:, :, None] == cols128[None, None, :]).astype(NPF8)
        in_maps.append({
            "xt": xt, "w1": w1h, "w2": w2h, "b1r": b1r, "b2r": b2r,
            "dinv_1": dv * (GS / WS), "dinv_3": dv / GS, "dinv_16": dv * GS,
            "eidx": eidx, "sful": sful,
        })
    return in_maps, nodes_of


# --------------------------------------------------------------- device side

def build(cfg, debug=False):
    nc = bacc.Bacc("TRN2", target_bir_lowering=False, debug=debug,
                   enable_asserts=False, num_devices=cfg.n_cores,
                   num_swdge_queues=4)
    T, NB = cfg.t, cfg.n_chunks
    FM, FMP, FO, KC, KC2 = cfg.f_mid, cfg.fmp, cfg.f_out, cfg.kc, cfg.kc2
    kb, bt, btmax, kbmax = cfg.kb, cfg.bt, cfg.btmax, cfg.kbmax
    nblk_tot = int(bt.sum())
    blk_off = np.zeros((T, NB), dtype=np.int64)
    run = 0
    for t in range(T):
        for cb in range(NB):
            blk_off[t, cb] = run
            run += kb[t, cb]
    tile_off = [int(blk_off[t, 0]) for t in range(T)]

    xt_d = nc.dram_tensor("xt", [P, KC, cfg.nsp], FP8, kind="ExternalInput").ap()
    w1_d = nc.dram_tensor("w1", [P, KC, FM], FP8, kind="ExternalInput").ap()
    w2_d = nc.dram_tensor("w2", [P, KC2, FO], F32, kind="ExternalInput").ap()
    b1_d = nc.dram_tensor("b1r", [P, FM], F32, kind="ExternalInput").ap()
    b2_d = nc.dram_tensor("b2r", [P, 8], F32, kind="ExternalInput").ap()
    dv1_d = nc.dram_tensor("dinv_1", [P, T], F32, kind="ExternalInput").ap()
    dv3_d = nc.dram_tensor("dinv_3", [P, T], F32, kind="ExternalInput").ap()
    dv16_d = nc.dram_tensor("dinv_16", [P, T], F32, kind="ExternalInput").ap()
    ei_d = nc.dram_tensor("eidx", [P, nblk_tot * 8], I16, kind="ExternalInput").ap()
    sf_d = nc.dram_tensor("sful", [P, nblk_tot, P], FP8, kind="ExternalInput").ap()
    out_d = nc.dram_tensor("out", [cfg.nsp, FO], F32, kind="ExternalOutput").ap()

    rg = [list(range(cfg.n_cores))]

    with tile.TileContext(nc) as tc:
        with tc.tile_pool(name="res", bufs=1) as res, \
             tc.tile_pool(name="dram", bufs=1, space="DRAM") as dram:
            g_local = dram.tile([cfg.nsp, FMP], FP8)
            g2_local = dram.tile([cfg.nsp, 256], FP8)
            if SPLIT_AG:
                g_full_a = dram.tile([cfg.na, FMP], FP8, addr_space="Shared")
                g_full_b = dram.tile([cfg.na, FMP], FP8, addr_space="Shared")
                g2_full_a = dram.tile([cfg.na, 256], FP8, addr_space="Shared")
                g2_full_b = dram.tile([cfg.na, 256], FP8, addr_space="Shared")
            else:
                g_full_a = dram.tile([cfg.ntot, FMP], FP8, addr_space="Shared")
                g_full_b = g_full_a
                g2_full_a = dram.tile([cfg.ntot, 256], FP8, addr_space="Shared")
                g2_full_b = g2_full_a

            w2_sb = res.tile([P, KC2, FO], F32)
            nc.sync.dma_start(out=w2_sb[:], in_=w2_d[:])
            b1_sb = res.tile([P, FM], F32)
            nc.sync.dma_start(out=b1_sb[:], in_=b1_d[:])
            b2_sb = res.tile([P, 8], F32)
            nc.sync.dma_start(out=b2_sb[:], in_=b2_d[:])
            dv1_sb = res.tile([P, T], F32)
            nc.sync.dma_start(out=dv1_sb[:], in_=dv1_d[:])
            dv3_sb = res.tile([P, T], F32)
            nc.sync.dma_start(out=dv3_sb[:], in_=dv3_d[:])
            dv16_sb = res.tile([P, T], F32)
            nc.sync.dma_start(out=dv16_sb[:], in_=dv16_d[:])
            ident = res.tile([P, P], F32)
            make_identity(nc, ident[:])

            # ---------------- phase 1: g = fp8(GS * dinv * (x @ W1))
            with tc.tile_pool(name="p1", bufs=3) as p1, \
                 tc.tile_pool(name="p1w", bufs=1) as p1w, \
                 tc.tile_pool(name="p1ps", bufs=2, space="PSUM") as p1ps:
                w1_sb = p1w.tile([P, KC, FM], FP8)
                nc.sync.dma_start(out=w1_sb[:], in_=w1_d[:])
                for t in range(T):
                    xtile = p1.tile([P, KC, P], FP8, tag="xtile")
                    nc.sync.dma_start(out=xtile[:], in_=xt_d[:, :, t * P:(t + 1) * P])
                    hp = p1ps.tile([P, FM], F32, tag="hp")
                    for f0 in range(0, FM, cfg.mm_free):
                        f1 = min(f0 + cfg.mm_free, FM)
                        for c in range(0, KC, 2):
                            nc.tensor.matmul(
                                out=hp[:, f0:f1], lhsT=xtile[:, c:c + 2, :],
                                rhs=w1_sb[:, c:c + 2, f0:f1],
                                start=(c == 0), stop=(c == KC - 2),
                                perf_mode=DR)
                    gt = p1.tile([P, FMP], FP8, tag="gt")
                    if t < 3:
                        nc.vector.memset(gt[:, FM:], 0.0)
                    nc.vector.tensor_scalar(
                        out=gt[:, :FM], in0=hp[:], scalar1=dv1_sb[:, t:t + 1],
                        scalar2=None, op0=ALU.mult)
                    nc.sync.dma_start(out=g_local[t * P:(t + 1) * P, :], in_=gt[:])

            # ---------------- phase 2: allgather g (two halves for overlap)
            if SPLIT_AG:
                nc.gpsimd.collective_compute(
                    "AllGather", ALU.bypass, replica_groups=rg,
                    ins=[g_local[0:cfg.ta * P, :]], outs=[g_full_a[:]])
                nc.gpsimd.collective_compute(
                    "AllGather", ALU.bypass, replica_groups=rg,
                    ins=[g_local[cfg.ta * P:, :]], outs=[g_full_b[:]])
            else:
                nc.gpsimd.collective_compute(
                    "AllGather", ALU.bypass, replica_groups=rg,
                    ins=[g_local[:]], outs=[g_full_a[:]])

            # ---------------- phase 3
            with tc.tile_pool(name="p3", bufs=2) as p3, \
                 tc.tile_pool(name="p3g", bufs=2) as p3g, \
                 tc.tile_pool(name="p3acc", bufs=2, space="PSUM") as p3acc, \
                 tc.tile_pool(name="p3ps", bufs=2, space="PSUM") as p3ps:
                nfs = (FM + cfg.mm_free - 1) // cfg.mm_free
                for t in range(T):
                    btt = int(bt[t])
                    o_t = tile_off[t]
                    eit = p3.tile([P, btmax * 8], I16, tag="eit")
                    nc.sync.dma_start(
                        out=eit[:, : btt * 8],
                        in_=ei_d[:, o_t * 8: (o_t + btt) * 8])
                    sst = p3.tile([P, btmax, P], FP8, tag="sst")
                    nc.sync.dma_start(
                        out=sst[:, :btt, :], in_=sf_d[:, o_t: o_t + btt, :])
                    gg = p3g.tile([P, btmax, FMP], FP8, tag="gg")
                    if t < 2:
                        nc.vector.memset(gg[:, :, :], 0.0)
                    for cb in range(NB):
                        kbb = int(kb[t, cb])
                        ni = kbb * P
                        co = int(blk_off[t, cb]) - o_t
                        rv = int(cfg.cmax[t, cb]) if DYN_CNT else ni
                        if SPLIT_AG:
                            ghalf = g_full_a if cb < NB // 2 else g_full_b
                            coff = (cb % (NB // 2)) * cfg.vc
                        else:
                            ghalf, coff = g_full_a, cb * cfg.vc
                        nc.gpsimd.dma_gather(
                            out_ap=gg[:, co:co + kbb, :],
                            in_ap=ghalf[coff:coff + cfg.vc, :],
                            idxs_ap=eit[:, co * 8: co * 8 + ni // 16],
                            num_idxs=ni, num_idxs_reg=rv, elem_size=FMP,
                            single_packet=(ni <= 1024), queue_num=(t * NB + cb) % 4)
                    acc = p3acc.tile([P, FM], F32, tag="acc")
                    for b in range(0, btt, 2):
                        for fi in range(nfs):
                            f0 = fi * cfg.mm_free
                            f1 = min(f0 + cfg.mm_free, FM)
                            nc.tensor.matmul(
                                out=acc[:, f0:f1], lhsT=sst[:, b:b + 2, :],
                                rhs=gg[:, b:b + 2, f0:f1],
                                start=(b == 0), stop=(b == btt - 2),
                                perf_mode=DR)
                    # epilogue: out1 = relu(dinv/GS*acc + b1)
                    t1 = p3.tile([P, FM], F32, tag="t1")
                    nc.vector.tensor_scalar(
                        out=t1[:], in0=acc[:], scalar1=dv3_sb[:, t:t + 1],
                        scalar2=None, op0=ALU.mult)
                    nc.vector.tensor_add(out=t1[:], in0=t1[:], in1=b1_sb[:])
                    nc.vector.tensor_scalar_max(out=t1[:], in0=t1[:], scalar1=0.0)
                    # g2T = W2^T @ t1^T
                    g2t = p3ps.tile([P, P], F32, tag="g2t")
                    for c in range(KC2):
                        f0 = c * P
                        cw = min(P, FM - f0)
                        tp = p3ps.tile([P, P], F32, tag="tp")
                        nc.tensor.transpose(
                            out=tp[:cw, :], in_=t1[:, f0:f0 + cw], identity=ident[:])
                        tps = p3.tile([P, P], F32, tag="tps")
                        nc.vector.tensor_copy(out=tps[:cw, :], in_=tp[:cw, :])
                        nc.tensor.matmul(
                            out=g2t[:FO, :], lhsT=w2_sb[:cw, c, :], rhs=tps[:cw, :],
                            start=(c == 0), stop=(c == KC2 - 1))
                    drp = p3ps.tile([P, P], F32, tag="tp")
                    nc.tensor.transpose(
                        out=drp[:], in_=dv16_sb[:, t:t + 1].to_broadcast([P, P]),
                        identity=ident[:])
                    dr = p3.tile([P, P], F32, tag="dr")
                    nc.vector.tensor_copy(out=dr[:], in_=drp[:])
                    g2s = p3.tile([P, P], F32, tag="g2s")
                    nc.vector.tensor_tensor(
                        out=g2s[:FO, :], in0=g2t[:FO, :], in1=dr[:FO, :], op=ALU.mult)
                    g2ntp = p3ps.tile([P, 8], F32, tag="tp")
                    nc.tensor.transpose(
                        out=g2ntp[:, :FO], in_=g2s[:FO, :], identity=ident[:FO, :FO])
                    g2o = p3.tile([P, 256], FP8, tag="g2o")
                    nc.vector.memset(g2o[:], 0.0)
                    nc.vector.tensor_copy(out=g2o[:, :FO], in_=g2ntp[:, :FO])
                    nc.sync.dma_start(
                        out=g2_local[t * P:(t + 1) * P, :], in_=g2o[:])

            # ---------------- phase 3.5: allgather g2 (two halves)
            if SPLIT_AG:
                nc.gpsimd.collective_compute(
                    "AllGather", ALU.bypass, replica_groups=rg,
                    ins=[g2_local[0:cfg.ta * P, :]], outs=[g2_full_a[:]])
                nc.gpsimd.collective_compute(
                    "AllGather", ALU.bypass, replica_groups=rg,
                    ins=[g2_local[cfg.ta * P:, :]], outs=[g2_full_b[:]])
            else:
                nc.gpsimd.collective_compute(
                    "AllGather", ALU.bypass, replica_groups=rg,
                    ins=[g2_local[:]], outs=[g2_full_a[:]])

            # ---------------- phase 4
            with tc.tile_pool(name="p4", bufs=2) as p4, \
                 tc.tile_pool(name="p4g", bufs=2) as p4g, \
                 tc.tile_pool(name="p4ps", bufs=2, space="PSUM") as p4ps:
                for t in range(T):
                    btt = int(bt[t])
                    o_t = tile_off[t]
                    eit = p4.tile([P, btmax * 8], I16, tag="eit4")
                    nc.sync.dma_start(
                        out=eit[:, : btt * 8],
                        in_=ei_d[:, o_t * 8: (o_t + btt) * 8])
                    sst = p4.tile([P, btmax, P], FP8, tag="sst4")
                    nc.sync.dma_start(
                        out=sst[:, :btt, :], in_=sf_d[:, o_t: o_t + btt, :])
                    gg2 = p4g.tile([P, btmax, 256], FP8, tag="gg2")
                    if t < 2:
                        nc.vector.memset(gg2[:, :, :], 0.0)
                    for cb in range(NB):
                        kbb = int(kb[t, cb])
                        ni = kbb * P
                        co = int(blk_off[t, cb]) - o_t
                        rv = int(cfg.cmax[t, cb]) if DYN_CNT else ni
                        if SPLIT_AG:
                            ghalf = g2_full_a if cb < NB // 2 else g2_full_b
                            coff = (cb % (NB // 2)) * cfg.vc
                        else:
                            ghalf, coff = g2_full_a, cb * cfg.vc
                        nc.gpsimd.dma_gather(
                            out_ap=gg2[:, co:co + kbb, :],
                            in_ap=ghalf[coff:coff + cfg.vc, :],
                            idxs_ap=eit[:, co * 8: co * 8 + ni // 16],
                            num_idxs=ni, num_idxs_reg=rv, elem_size=256,
                            single_packet=(ni <= 1024), queue_num=(t * NB + cb) % 4)
                    acc2 = p4ps.tile([P, P], F32, tag="acc2")
                    for b in range(0, btt, 2):
                        nc.tensor.matmul(
                            out=acc2[:8, :], lhsT=gg2[:, b:b + 2, :8],
                            rhs=sst[:, b:b + 2, :],
                            start=(b == 0), stop=(b == btt - 2),
                            perf_mode=DR)
                    t2s = p4.tile([P, P], F32, tag="t2s")
                    nc.vector.tensor_copy(out=t2s[:8, :], in_=acc2[:8, :])
                    t2ntp = p4ps.tile([P, 8], F32, tag="t2ntp")
                    nc.tensor.transpose(
                        out=t2ntp[:, :8], in_=t2s[:8, :], identity=ident[:8, :8])
                    tf = p4.tile([P, 8], F32, tag="tf")
                    nc.vector.tensor_scalar(
                        out=tf[:], in0=t2ntp[:], scalar1=dv3_sb[:, t:t + 1],
                        scalar2=None, op0=ALU.mult)
                    nc.vector.tensor_add(out=tf[:], in0=tf[:], in1=b2_sb[:])
                    nm = p4.tile([P, 1], F32, tag="nm")
                    nc.vector.tensor_reduce(
                        out=nm[:], in_=tf[:, :FO], axis=AX.X, op=ALU.max, negate=True)
                    ex = p4.tile([P, 8], F32, tag="ex")
                    se = p4.tile([P, 1], F32, tag="se")
                    nc.scalar.activation(
                        out=ex[:, :FO], in_=tf[:, :FO], func=ACT.Exp,
                        bias=nm[:, :1], scale=1.0, accum_out=se[:, :1])
                    lse = p4.tile([P, 1], F32, tag="lse")
                    nc.scalar.activation(out=lse[:], in_=se[:], func=ACT.Ln)
                    of = p4.tile([P, 8], F32, tag="of")
                    nc.vector.tensor_scalar(
                        out=of[:, :FO], in0=tf[:, :FO], scalar1=nm[:, :1],
                        scalar2=lse[:, :1], op0=ALU.add, op1=ALU.subtract)
                    nc.sync.dma_start(out=out_d[t * P:(t + 1) * P, :], in_=of[:, :FO])

    nc.compile()
    return nc


# ------------------------------------------------------------------ runner

def _run(inputs, cfg=None, trace=False, trace_kwargs=None):
    cfg = cfg or Cfg()
    in_maps, nodes_of = preprocess(
        inputs["x"], inputs["edge_index"], inputs["W1"], inputs["b1"],
        inputs["W2"], inputs["b2"], cfg)
    nc = build(cfg)
    res = bass_utils.run_bass_kernel_spmd(
        nc, in_maps, core_ids=list(range(cfg.n_cores)), trace=trace,
        **(trace_kwargs or {}))
    out = np.zeros((cfg.n_nodes, cfg.f_out), dtype=np.float32)
    for c in range(cfg.n_cores):
        oc = res.results[c]["out"]
        nv = nodes_of[c]
        valid = nv >= 0
        out[nv[valid]] = oc[valid]
    return out, res


def kernel(**inputs):
    out, _ = _run(inputs)
    return out


# revision 46
# speedup vs baseline: 1.0262x; 1.0262x over previous
"""Two-layer GCN (GCNConv -> ReLU -> GCNConv -> log_softmax) on 8 Trainium2
NeuronCores.

Strategy (graph/data parallel node partitioning), rev A (fp8):
  * Destination nodes are dealt round-robin by in-degree across cores and
    tiles (host-side) so per-(core,tile,chunk) edge buckets are balanced.
  * Phase 1: each core computes g = fp8(16 * dinv * (x_shard @ W1)) for its
    own nodes via fp8 DoubleRow matmuls (x in e4m3, W1*64 in e4m3, fp32 PSUM),
    stores its g-table shard as fp8 rows padded to 1024B.
  * Phase 2: AllGather the fp8 g table (103 MB full table).
  * Phase 3: per dst tile, `dma_gather` pulls 1024B source rows for all
    in-edges (edges bucketed by table quarter-chunk for int16 indices);
    a host-prebuilt fp8 selection matrix S is streamed from HBM and the
    per-destination segment-sum becomes fp8 DoubleRow PE matmuls (block
    pairs) accumulated in PSUM. Epilogue: out1 = relu(dinv/16*acc + b1);
    g2 = dinv * (out1 @ W2) via PE transposes; g2 stored fp16 in 256B rows.
  * Phase 3.5: AllGather g2 (fp8, 256B rows).
  * Phase 4: gather 256B g2 rows per edge, fp8 DoubleRow matmul against the
    same streamed S, then dinv, b2 and log_softmax.

  The global table is laid out as [all cores' tiles 0..T/2-1 | tiles
  T/2..T-1] so each AllGather splits into two collectives and phase 3/4
  chunk-0/1 work overlaps the second half's transfer.  Bucket padding uses
  negative indices (skipped by the gather ucode per-core), with at least 16
  real descriptors per call to keep the completion semaphore sane.

Self-contained: hardcodes shapes; only needs the container toolchain at
/opt/trn_rl_repo.
"""

import os
import sys

for _p in ("/opt/trn_rl_repo",):
    if _p not in sys.path:
        sys.path.insert(0, _p)

import ml_dtypes
import numpy as np

import concourse.bacc as bacc
import concourse.bass as bass
import concourse.tile as tile
from concourse import bass_utils, mybir
from concourse.masks import make_identity

P = 128
FP16 = mybir.dt.float16
FP8 = mybir.dt.float8e4
F32 = mybir.dt.float32
I16 = mybir.dt.int16
I32 = mybir.dt.int32
AX = mybir.AxisListType
ALU = mybir.AluOpType
ACT = mybir.ActivationFunctionType
DR = mybir.MatmulPerfMode.DoubleRow
NPF8 = ml_dtypes.float8_e4m3fn
SPLIT_AG = bool(int(os.environ.get("GCN_SPLITAG", "1")))
DYN_CNT = bool(int(os.environ.get("GCN_DYN", "1")))

GS = 16.0     # g-table fp8 scale: stored g_q = g * GS
WS = 64.0     # W1 fp8 scale: stored w_q = W1 * WS


class Cfg:
    def __init__(self, n_nodes=100000, n_cores=8, f_in=1433, f_mid=789, f_out=7,
                 n_chunks=4, mm_free=512):
        self.n_nodes = n_nodes
        self.n_cores = n_cores
        self.f_in = f_in
        self.kc = (f_in + P - 1) // P          # k-chunks for layer-1 matmul
        assert self.kc % 2 == 0
        self.f_mid = f_mid
        self.fmp = ((f_mid + 255) // 256) * 256   # fp8 row padded to 256B: 1024
        self.kc2 = (f_mid + P - 1) // P        # k-chunks for layer-2 matmul
        self.f_out = f_out
        self.ns = n_nodes // n_cores           # nodes per shard (pre-pad)
        assert self.ns * n_cores == n_nodes
        self.t = (self.ns + P - 1) // P        # dst tiles per core
        assert self.t % 2 == 0
        self.ta = self.t // 2                  # tiles in table half A
        self.nsp = self.t * P                  # padded shard size
        self.ntot = self.nsp * n_cores         # padded global table rows
        self.na = self.ntot // 2               # rows in table half A
        self.n_chunks = n_chunks               # int16 table chunks
        assert self.ntot % n_chunks == 0
        self.vc = self.ntot // n_chunks        # rows per chunk
        assert self.vc < 32768
        self.mm_free = mm_free
        # set by preprocess:
        self.kb = None                         # [t][cb] blocks per bucket
        self.bt = None                         # [t] total blocks per tile
        self.btmax = None
        self.kbmax = None


# ----------------------------------------------------------------- host side

def preprocess(x, edge_index, W1, b1, W2, b2, cfg):
    """Shard + permute nodes, bucket edges by (dst tile, src chunk)."""
    N, C = cfg.n_nodes, cfg.n_cores
    src = np.asarray(edge_index[0], dtype=np.int64)
    dst = np.asarray(edge_index[1], dtype=np.int64)
    loop = np.arange(N, dtype=np.int64)
    src = np.concatenate([src, loop])
    dst = np.concatenate([dst, loop])

    deg = np.bincount(dst, minlength=N).astype(np.float64)
    dinv = (1.0 / np.sqrt(deg)).astype(np.float32)

    # deal nodes round-robin by in-degree across cores, then snake across
    # tiles within each core, to balance (core, tile, chunk) bucket counts.
    indeg = np.bincount(dst, minlength=N)
    order_glob = np.argsort(-indeg, kind="stable")
    shard_of = np.zeros(N, dtype=np.int64)
    node_tile = np.zeros(N, dtype=np.int64)
    node_col = np.zeros(N, dtype=np.int64)
    pg = np.zeros(N, dtype=np.int64)
    nodes_of = []
    snake = np.concatenate([np.arange(cfg.t), np.arange(cfg.t)[::-1]])
    tiles_seq = np.tile(snake, (P + 1) // 2 + 1)[: cfg.nsp]
    for c in range(C):
        order = order_glob[c::C]               # this core's nodes, by degree
        shard_of[order] = c
        tl = tiles_seq[: cfg.ns]
        node_tile[order] = tl
        pos = np.argsort(tl, kind="stable")
        cols = np.empty(cfg.ns, dtype=np.int64)
        tile_sorted = tl[pos]
        start = np.searchsorted(tile_sorted, np.arange(cfg.t))
        cols[pos] = np.arange(cfg.ns) - start[tile_sorted]
        node_col[order] = cols
        if SPLIT_AG:
            half_b = tl >= cfg.ta
            pg[order] = np.where(
                half_b,
                cfg.na + c * cfg.ta * P + (tl - cfg.ta) * P + cols,
                c * cfg.ta * P + tl * P + cols)
        else:
            pg[order] = c * cfg.nsp + tl * P + cols
        nv = np.full(cfg.nsp, -1, dtype=np.int64)
        nv[tl * P + cols] = order
        nodes_of.append(nv)

    # ---- bucket edges by (core, dst tile, src chunk)
    e_shard = shard_of[dst]
    e_tile = node_tile[dst]
    e_src_pg = pg[src]
    e_chunk = e_src_pg // cfg.vc
    e_dcol = node_col[dst]
    NB = cfg.n_chunks
    counts = np.zeros((C, cfg.t, NB), dtype=np.int64)
    np.add.at(counts, (e_shard, e_tile, e_chunk), 1)
    kb = ((counts.max(axis=0) + P - 1) // P).astype(np.int64)   # [t, NB]
    kb = np.maximum(kb, 1)
    # shared (max-over-cores) real index count per bucket; trailing slots up
    # to kb*128 are -1 and skipped by the gather ucode
    cfg.cmax = np.maximum(counts.max(axis=0), 16).astype(np.int64)
    # per-tile block totals must be even for DoubleRow pairing
    odd = kb.sum(axis=1) % 2 == 1
    kb[odd, NB - 1] += 1
    cfg.kb = kb
    cfg.bt = kb.sum(axis=1)                   # [t]
    cfg.btmax = int(cfg.bt.max())
    if cfg.btmax % 2:
        cfg.btmax += 1
    cfg.kbmax = int(kb.max())
    nblk_tot = int(cfg.bt.sum())

    order_all = np.lexsort((e_src_pg, e_chunk, e_tile, e_shard))
    s_sorted = (e_src_pg - e_chunk * cfg.vc)[order_all].astype(np.int16)
    d_sorted = e_dcol[order_all].astype(np.int64)
    key = (e_shard * cfg.t + e_tile)[order_all] * NB + e_chunk[order_all]
    bounds = np.searchsorted(key, np.arange(C * cfg.t * NB + 1))

    # block offsets per (t, cb)
    blk_off = np.zeros((cfg.t, NB), dtype=np.int64)
    run = 0
    for t in range(cfg.t):
        for cb in range(NB):
            blk_off[t, cb] = run
            run += kb[t, cb]

    xpad = np.zeros((cfg.kc * P, N), dtype=NPF8)
    xq = np.clip(np.asarray(x, dtype=np.float32), -240, 240)
    xpad[: cfg.f_in, :] = xq.T.astype(NPF8)
    w1h = np.zeros((P, cfg.kc, cfg.f_mid), dtype=NPF8)
    w1t = np.zeros((cfg.kc * P, cfg.f_mid), dtype=np.float32)
    w1t[: cfg.f_in] = np.clip(np.asarray(W1, dtype=np.float32) * WS, -240, 240)
    w1h[:] = w1t.reshape(cfg.kc, P, cfg.f_mid).transpose(1, 0, 2).astype(NPF8)
    w2h = np.zeros((P, cfg.kc2, cfg.f_out), dtype=np.float32)
    w2t = np.zeros((cfg.kc2 * P, cfg.f_out), dtype=np.float32)
    w2t[: cfg.f_mid] = np.asarray(W2, dtype=np.float32)
    w2h[:] = w2t.reshape(cfg.kc2, P, cfg.f_out).transpose(1, 0, 2)
    b1r = np.tile(np.asarray(b1, dtype=np.float32)[None, :], (P, 1))
    b2r = np.zeros((P, 8), dtype=np.float32)
    b2r[:, : cfg.f_out] = np.asarray(b2, dtype=np.float32)[None, :]

    cols128 = np.arange(P, dtype=np.int64)
    in_maps = []
    for c in range(C):
        nv = nodes_of[c]
        valid = nv >= 0
        xs = np.zeros((cfg.kc * P, cfg.nsp), dtype=NPF8)
        xs[:, valid] = xpad[:, nv[valid]]
        xt = np.ascontiguousarray(xs.reshape(cfg.kc, P, cfg.nsp).transpose(1, 0, 2))
        dvt = np.zeros(cfg.nsp, dtype=np.float32)
        dvt[valid] = dinv[nv[valid]]
        dv = np.ascontiguousarray(dvt.reshape(cfg.t, P).T)
        # idx: per (t, cb): kb*128 int16, idx j at [j%16, off*8 + j//16]
        eidx = np.zeros((P, nblk_tot * 8), dtype=np.int16)
        # S: per block b, S[p, b, col] = 1 if edge slot (b*128+p) -> dst col
        sful = np.zeros((P, nblk_tot, P), dtype=NPF8)
        for t in range(cfg.t):
            for cb in range(NB):
                lo = bounds[(c * cfg.t + t) * NB + cb]
                hi = bounds[(c * cfg.t + t) * NB + cb + 1]
                cnt = hi - lo
                nsl = int(kb[t, cb]) * P
                off = int(blk_off[t, cb])
                # trailing -1 idxs are skipped by the gather ucode; keep at
                # least 16 non-negative so every SDMA engine gets a desc
                cmv = int(cfg.cmax[t, cb])
                ai = np.full(nsl, -1 if DYN_CNT else 0, dtype=np.int16)
                if DYN_CNT:
                    ai[:cmv] = 0
                ai[:cnt] = s_sorted[lo:hi]
                eidx[:, off * 8: off * 8 + nsl // 16] = np.tile(
                    ai.reshape(nsl // 16, 16).T, (8, 1))
                ad = np.full(nsl, -1, dtype=np.int64)
                ad[:cnt] = d_sorted[lo:hi]
                blkd = ad.reshape(int(kb[t, cb]), P)      # [kb, 128] dst cols
                sful[:, off: off + int(kb[t, cb]), :] = (
                    blkd.T[:, :, None] == cols128[None, None, :]).astype(NPF8)
        in_maps.append({
            "xt": xt, "w1": w1h, "w2": w2h, "b1r": b1r, "b2r": b2r,
            "dinv_1": dv * (GS / WS), "dinv_3": dv / GS, "dinv_16": dv * GS,
            "eidx": eidx, "sful": sful,
        })
    return in_maps, nodes_of


# --------------------------------------------------------------- device side

def build(cfg, debug=False):
    nc = bacc.Bacc("TRN2", target_bir_lowering=False, debug=debug,
                   enable_asserts=False, num_devices=cfg.n_cores,
                   num_swdge_queues=4)
    T, NB = cfg.t, cfg.n_chunks
    FM, FMP, FO, KC, KC2 = cfg.f_mid, cfg.fmp, cfg.f_out, cfg.kc, cfg.kc2
    kb, bt, btmax, kbmax = cfg.kb, cfg.bt, cfg.btmax, cfg.kbmax
    nblk_tot = int(bt.sum())
    blk_off = np.zeros((T, NB), dtype=np.int64)
    run = 0
    for t in range(T):
        for cb in range(NB):
            blk_off[t, cb] = run
            run += kb[t, cb]
    tile_off = [int(blk_off[t, 0]) for t in range(T)]

    xt_d = nc.dram_tensor("xt", [P, KC, cfg.nsp], FP8, kind="ExternalInput").ap()
    w1_d = nc.dram_tensor("w1", [P, KC, FM], FP8, kind="ExternalInput").ap()
    w2_d = nc.dram_tensor("w2", [P, KC2, FO], F32, kind="ExternalInput").ap()
    b1_d = nc.dram_tensor("b1r", [P, FM], F32, kind="ExternalInput").ap()
    b2_d = nc.dram_tensor("b2r", [P, 8], F32, kind="ExternalInput").ap()
    dv1_d = nc.dram_tensor("dinv_1", [P, T], F32, kind="ExternalInput").ap()
    dv3_d = nc.dram_tensor("dinv_3", [P, T], F32, kind="ExternalInput").ap()
    dv16_d = nc.dram_tensor("dinv_16", [P, T], F32, kind="ExternalInput").ap()
    ei_d = nc.dram_tensor("eidx", [P, nblk_tot * 8], I16, kind="ExternalInput").ap()
    sf_d = nc.dram_tensor("sful", [P, nblk_tot, P], FP8, kind="ExternalInput").ap()
    out_d = nc.dram_tensor("out", [cfg.nsp, FO], F32, kind="ExternalOutput").ap()

    rg = [list(range(cfg.n_cores))]

    with tile.TileContext(nc) as tc:
        with tc.tile_pool(name="res", bufs=1) as res, \
             tc.tile_pool(name="dram", bufs=1, space="DRAM") as dram:
            g_local = dram.tile([cfg.nsp, FMP], FP8)
            g2_local = dram.tile([cfg.nsp, 256], FP8)
            if SPLIT_AG:
                g_full_a = dram.tile([cfg.na, FMP], FP8, addr_space="Shared")
                g_full_b = dram.tile([cfg.na, FMP], FP8, addr_space="Shared")
                g2_full_a = dram.tile([cfg.na, 256], FP8, addr_space="Shared")
                g2_full_b = dram.tile([cfg.na, 256], FP8, addr_space="Shared")
            else:
                g_full_a = dram.tile([cfg.ntot, FMP], FP8, addr_space="Shared")
                g_full_b = g_full_a
                g2_full_a = dram.tile([cfg.ntot, 256], FP8, addr_space="Shared")
                g2_full_b = g2_full_a

            w2_sb = res.tile([P, KC2, FO], F32)
            nc.sync.dma_start(out=w2_sb[:], in_=w2_d[:])
            b1_sb = res.tile([P, FM], F32)
            nc.sync.dma_start(out=b1_sb[:], in_=b1_d[:])
            b2_sb = res.tile([P, 8], F32)
            nc.sync.dma_start(out=b2_sb[:], in_=b2_d[:])
            dv1_sb = res.tile([P, T], F32)
            nc.sync.dma_start(out=dv1_sb[:], in_=dv1_d[:])
            dv3_sb = res.tile([P, T], F32)
            nc.sync.dma_start(out=dv3_sb[:], in_=dv3_d[:])
            dv16_sb = res.tile([P, T], F32)
            nc.sync.dma_start(out=dv16_sb[:], in_=dv16_d[:])
            ident = res.tile([P, P], F32)
            make_identity(nc, ident[:])

            # ---------------- phase 1: g = fp8(GS * dinv * (x @ W1))
            with tc.tile_pool(name="p1", bufs=3) as p1, \
                 tc.tile_pool(name="p1w", bufs=1) as p1w, \
                 tc.tile_pool(name="p1ps", bufs=2, space="PSUM") as p1ps:
                w1_sb = p1w.tile([P, KC, FM], FP8)
                nc.sync.dma_start(out=w1_sb[:], in_=w1_d[:])
                for t in range(T):
                    xtile = p1.tile([P, KC, P], FP8, tag="xtile")
                    nc.sync.dma_start(out=xtile[:], in_=xt_d[:, :, t * P:(t + 1) * P])
                    hp = p1ps.tile([P, FM], F32, tag="hp")
                    for f0 in range(0, FM, cfg.mm_free):
                        f1 = min(f0 + cfg.mm_free, FM)
                        for c in range(0, KC, 2):
                            nc.tensor.matmul(
                                out=hp[:, f0:f1], lhsT=xtile[:, c:c + 2, :],
                                rhs=w1_sb[:, c:c + 2, f0:f1],
                                start=(c == 0), stop=(c == KC - 2),
                                perf_mode=DR)
                    gt = p1.tile([P, FMP], FP8, tag="gt")
                    if t < 3:
                        nc.vector.memset(gt[:, FM:], 0.0)
                    nc.vector.tensor_scalar(
                        out=gt[:, :FM], in0=hp[:], scalar1=dv1_sb[:, t:t + 1],
                        scalar2=None, op0=ALU.mult)
                    nc.sync.dma_start(out=g_local[t * P:(t + 1) * P, :], in_=gt[:])

            # ---------------- phase 2: allgather g (two halves for overlap)
            if SPLIT_AG:
                nc.gpsimd.collective_compute(
                    "AllGather", ALU.bypass, replica_groups=rg,
                    ins=[g_local[0:cfg.ta * P, :]], outs=[g_full_a[:]])
                nc.gpsimd.collective_compute(
                    "AllGather", ALU.bypass, replica_groups=rg,
                    ins=[g_local[cfg.ta * P:, :]], outs=[g_full_b[:]])
            else:
                nc.gpsimd.collective_compute(
                    "AllGather", ALU.bypass, replica_groups=rg,
                    ins=[g_local[:]], outs=[g_full_a[:]])

            # ---------------- phase 3
            with tc.tile_pool(name="p3", bufs=3) as p3, \
                 tc.tile_pool(name="p3g", bufs=3) as p3g, \
                 tc.tile_pool(name="p3acc", bufs=2, space="PSUM") as p3acc, \
                 tc.tile_pool(name="p3ps", bufs=2, space="PSUM") as p3ps:
                nfs = (FM + cfg.mm_free - 1) // cfg.mm_free
                for t in range(T):
                    btt = int(bt[t])
                    o_t = tile_off[t]
                    eit = p3.tile([P, btmax * 8], I16, tag="eit")
                    nc.sync.dma_start(
                        out=eit[:, : btt * 8],
                        in_=ei_d[:, o_t * 8: (o_t + btt) * 8])
                    sst = p3.tile([P, btmax, P], FP8, tag="sst")
                    nc.sync.dma_start(
                        out=sst[:, :btt, :], in_=sf_d[:, o_t: o_t + btt, :])
                    gg = p3g.tile([P, btmax, FMP], FP8, tag="gg")
                    if t < 3:
                        nc.vector.memset(gg[:, :, :], 0.0)
                    for cb in range(NB):
                        kbb = int(kb[t, cb])
                        ni = kbb * P
                        co = int(blk_off[t, cb]) - o_t
                        rv = int(cfg.cmax[t, cb]) if DYN_CNT else ni
                        if SPLIT_AG:
                            ghalf = g_full_a if cb < NB // 2 else g_full_b
                            coff = (cb % (NB // 2)) * cfg.vc
                        else:
                            ghalf, coff = g_full_a, cb * cfg.vc
                        nc.gpsimd.dma_gather(
                            out_ap=gg[:, co:co + kbb, :],
                            in_ap=ghalf[coff:coff + cfg.vc, :],
                            idxs_ap=eit[:, co * 8: co * 8 + ni // 16],
                            num_idxs=ni, num_idxs_reg=rv, elem_size=FMP,
                            single_packet=(ni <= 1024), queue_num=(t * NB + cb) % 4)
                    acc = p3acc.tile([P, FM], F32, tag="acc")
                    for b in range(0, btt, 2):
                        for fi in range(nfs):
                            f0 = fi * cfg.mm_free
                            f1 = min(f0 + cfg.mm_free, FM)
                            nc.tensor.matmul(
                                out=acc[:, f0:f1], lhsT=sst[:, b:b + 2, :],
                                rhs=gg[:, b:b + 2, f0:f1],
                                start=(b == 0), stop=(b == btt - 2),
                                perf_mode=DR)
                    # epilogue: out1 = relu(dinv/GS*acc + b1)
                    t1 = p3.tile([P, FM], F32, tag="t1")
                    nc.vector.tensor_scalar(
                        out=t1[:], in0=acc[:], scalar1=dv3_sb[:, t:t + 1],
                        scalar2=None, op0=ALU.mult)
                    nc.vector.tensor_add(out=t1[:], in0=t1[:], in1=b1_sb[:])
                    nc.vector.tensor_scalar_max(out=t1[:], in0=t1[:], scalar1=0.0)
                    # g2T = W2^T @ t1^T
                    g2t = p3ps.tile([P, P], F32, tag="g2t")
                    for c in range(KC2):
                        f0 = c * P
                        cw = min(P, FM - f0)
                        tp = p3ps.tile([P, P], F32, tag="tp")
                        nc.tensor.transpose(
                            out=tp[:cw, :], in_=t1[:, f0:f0 + cw], identity=ident[:])
                        tps = p3.tile([P, P], F32, tag="tps")
                        nc.vector.tensor_copy(out=tps[:cw, :], in_=tp[:cw, :])
                        nc.tensor.matmul(
                            out=g2t[:FO, :], lhsT=w2_sb[:cw, c, :], rhs=tps[:cw, :],
                            start=(c == 0), stop=(c == KC2 - 1))
                    drp = p3ps.tile([P, P], F32, tag="tp")
                    nc.tensor.transpose(
                        out=drp[:], in_=dv16_sb[:, t:t + 1].to_broadcast([P, P]),
                        identity=ident[:])
                    dr = p3.tile([P, P], F32, tag="dr")
                    nc.vector.tensor_copy(out=dr[:], in_=drp[:])
                    g2s = p3.tile([P, P], F32, tag="g2s")
                    nc.vector.tensor_tensor(
                        out=g2s[:FO, :], in0=g2t[:FO, :], in1=dr[:FO, :], op=ALU.mult)
                    g2ntp = p3ps.tile([P, 8], F32, tag="tp")
                    nc.tensor.transpose(
                        out=g2ntp[:, :FO], in_=g2s[:FO, :], identity=ident[:FO, :FO])
                    g2o = p3.tile([P, 256], FP8, tag="g2o")
                    nc.vector.memset(g2o[:], 0.0)
                    nc.vector.tensor_copy(out=g2o[:, :FO], in_=g2ntp[:, :FO])
                    nc.sync.dma_start(
                        out=g2_local[t * P:(t + 1) * P, :], in_=g2o[:])

            # ---------------- phase 3.5: allgather g2 (two halves)
            if SPLIT_AG:
                nc.gpsimd.collective_compute(
                    "AllGather", ALU.bypass, replica_groups=rg,
                    ins=[g2_local[0:cfg.ta * P, :]], outs=[g2_full_a[:]])
                nc.gpsimd.collective_compute(
                    "AllGather", ALU.bypass, replica_groups=rg,
                    ins=[g2_local[cfg.ta * P:, :]], outs=[g2_full_b[:]])
            else:
                nc.gpsimd.collective_compute(
                    "AllGather", ALU.bypass, replica_groups=rg,
                    ins=[g2_local[:]], outs=[g2_full_a[:]])

            # ---------------- phase 4
            with tc.tile_pool(name="p4", bufs=3) as p4, \
                 tc.tile_pool(name="p4g", bufs=3) as p4g, \
                 tc.tile_pool(name="p4ps", bufs=2, space="PSUM") as p4ps:
                for t in range(T):
                    btt = int(bt[t])
                    o_t = tile_off[t]
                    eit = p4.tile([P, btmax * 8], I16, tag="eit4")
                    nc.sync.dma_start(
                        out=eit[:, : btt * 8],
                        in_=ei_d[:, o_t * 8: (o_t + btt) * 8])
                    sst = p4.tile([P, btmax, P], FP8, tag="sst4")
                    nc.sync.dma_start(
                        out=sst[:, :btt, :], in_=sf_d[:, o_t: o_t + btt, :])
                    gg2 = p4g.tile([P, btmax, 256], FP8, tag="gg2")
                    if t < 3:
                        nc.vector.memset(gg2[:, :, :], 0.0)
                    for cb in range(NB):
                        kbb = int(kb[t, cb])
                        ni = kbb * P
                        co = int(blk_off[t, cb]) - o_t
                        rv = int(cfg.cmax[t, cb]) if DYN_CNT else ni
                        if SPLIT_AG:
                            ghalf = g2_full_a if cb < NB // 2 else g2_full_b
                            coff = (cb % (NB // 2)) * cfg.vc
                        else:
                            ghalf, coff = g2_full_a, cb * cfg.vc
                        nc.gpsimd.dma_gather(
                            out_ap=gg2[:, co:co + kbb, :],
                            in_ap=ghalf[coff:coff + cfg.vc, :],
                            idxs_ap=eit[:, co * 8: co * 8 + ni // 16],
                            num_idxs=ni, num_idxs_reg=rv, elem_size=256,
                            single_packet=(ni <= 1024), queue_num=(t * NB + cb) % 4)
                    acc2 = p4ps.tile([P, P], F32, tag="acc2")
                    for b in range(0, btt, 2):
                        nc.tensor.matmul(
                            out=acc2[:8, :], lhsT=gg2[:, b:b + 2, :8],
                            rhs=sst[:, b:b + 2, :],
                            start=(b == 0), stop=(b == btt - 2),
                            perf_mode=DR)
                    t2s = p4.tile([P, P], F32, tag="t2s")
                    nc.vector.tensor_copy(out=t2s[:8, :], in_=acc2[:8, :])
                    t2ntp = p4ps.tile([P, 8], F32, tag="t2ntp")
                    nc.tensor.transpose(
                        out=t2ntp[:, :8], in_=t2s[:8, :], identity=ident[:8, :8])
                    tf = p4.tile([P, 8], F32, tag="tf")
                    nc.vector.tensor_scalar(
                        out=tf[:], in0=t2ntp[:], scalar1=dv3_sb[:, t:t + 1],
                        scalar2=None, op0=ALU.mult)
                    nc.vector.tensor_add(out=tf[:], in0=tf[:], in1=b2_sb[:])
                    nm = p4.tile([P, 1], F32, tag="nm")
                    nc.vector.tensor_reduce(
                        out=nm[:], in_=tf[:, :FO], axis=AX.X, op=ALU.max, negate=True)
                    ex = p4.tile([P, 8], F32, tag="ex")
                    se = p4.tile([P, 1], F32, tag="se")
                    nc.scalar.activation(
                        out=ex[:, :FO], in_=tf[:, :FO], func=ACT.Exp,
                        bias=nm[:, :1], scale=1.0, accum_out=se[:, :1])
                    lse = p4.tile([P, 1], F32, tag="lse")
                    nc.scalar.activation(out=lse[:], in_=se[:], func=ACT.Ln)
                    of = p4.tile([P, 8], F32, tag="of")
                    nc.vector.tensor_scalar(
                        out=of[:, :FO], in0=tf[:, :FO], scalar1=nm[:, :1],
                        scalar2=lse[:, :1], op0=ALU.add, op1=ALU.subtract)
                    nc.sync.dma_start(out=out_d[t * P:(t + 1) * P, :], in_=of[:, :FO])

    nc.compile()
    return nc


# ------------------------------------------------------------------ runner

def _run(inputs, cfg=None, trace=False, trace_kwargs=None):
    cfg = cfg or Cfg()
    in_maps, nodes_of = preprocess(
        inputs["x"], inputs["edge_index"], inputs["W1"], inputs["b1"],
        inputs["W2"], inputs["b2"], cfg)
    nc = build(cfg)
    res = bass_utils.run_bass_kernel_spmd(
        nc, in_maps, core_ids=list(range(cfg.n_cores)), trace=trace,
        **(trace_kwargs or {}))
    out = np.zeros((cfg.n_nodes, cfg.f_out), dtype=np.float32)
    for c in range(cfg.n_cores):
        oc = res.results[c]["out"]
        nv = nodes_of[c]
        valid = nv >= 0
        out[nv[valid]] = oc[valid]
    return out, res


def kernel(**inputs):
    out, _ = _run(inputs)
    return out


# revision 49
# speedup vs baseline: 1.0349x; 1.0084x over previous
"""Two-layer GCN (GCNConv -> ReLU -> GCNConv -> log_softmax) on 8 Trainium2
NeuronCores.

Strategy (graph/data parallel node partitioning), fp8 revision:
  * Destination nodes are dealt round-robin by in-degree across cores and
    tiles (host-side) so per-(core,tile,chunk) edge buckets are balanced.
  * Phase 1: each core computes g = fp8(16 * dinv * (x_shard @ W1)) for its
    own nodes via fp8 DoubleRow matmuls (x in e4m3, W1*64 in e4m3, fp32 PSUM),
    stores its g-table shard as fp8 rows padded to 1024B.
  * Phase 2: AllGather the fp8 g table (103 MB full table).
  * Phase 3: per dst tile, `dma_gather` pulls 1024B source rows for all
    in-edges (edges bucketed by table quarter-chunk for int16 indices);
    a host-prebuilt fp8 selection matrix S is streamed from HBM and the
    per-destination segment-sum becomes fp8 DoubleRow PE matmuls (block
    pairs) accumulated in PSUM. Epilogue: out1 = relu(dinv/16*acc + b1);
    g2 = dinv * (out1 @ W2) via PE transposes; g2 stored fp16 in 256B rows.
  * Phase 3.5: AllGather g2 (fp8, 256B rows).
  * Phase 4: gather 256B g2 rows per edge, fp8 DoubleRow matmul against the
    same streamed S, then dinv, b2 and log_softmax.

  The global table is laid out as [all cores' tiles 0..T/2-1 | tiles
  T/2..T-1] so each AllGather splits into two collectives and phase 3/4
  chunk-0/1 work overlaps the second half's transfer.  Bucket padding uses
  negative indices (skipped by the gather ucode per-core), with at least 16
  real descriptors per call to keep the completion semaphore sane.

Self-contained: hardcodes shapes; only needs the container toolchain at
/opt/trn_rl_repo.
"""

import os
import sys

for _p in ("/opt/trn_rl_repo",):
    if _p not in sys.path:
        sys.path.insert(0, _p)

import ml_dtypes
import numpy as np

import concourse.bacc as bacc
import concourse.bass as bass
import concourse.tile as tile
from concourse import bass_utils, mybir
from concourse.masks import make_identity

P = 128
FP16 = mybir.dt.float16
FP8 = mybir.dt.float8e4
F32 = mybir.dt.float32
I16 = mybir.dt.int16
I32 = mybir.dt.int32
AX = mybir.AxisListType
ALU = mybir.AluOpType
ACT = mybir.ActivationFunctionType
DR = mybir.MatmulPerfMode.DoubleRow
NPF8 = ml_dtypes.float8_e4m3fn
SPLIT_AG = bool(int(os.environ.get("GCN_SPLITAG", "1")))
DYN_CNT = bool(int(os.environ.get("GCN_DYN", "1")))

GS = 16.0     # g-table fp8 scale: stored g_q = g * GS
WS = 64.0     # W1 fp8 scale: stored w_q = W1 * WS


class Cfg:
    def __init__(self, n_nodes=100000, n_cores=8, f_in=1433, f_mid=789, f_out=7,
                 n_chunks=4, mm_free=512):
        self.n_nodes = n_nodes
        self.n_cores = n_cores
        self.f_in = f_in
        self.kc = (f_in + P - 1) // P          # k-chunks for layer-1 matmul
        assert self.kc % 2 == 0
        self.f_mid = f_mid
        self.fmp = ((f_mid + 255) // 256) * 256   # fp8 row padded to 256B: 1024
        self.kc2 = (f_mid + P - 1) // P        # k-chunks for layer-2 matmul
        self.f_out = f_out
        self.ns = n_nodes // n_cores           # nodes per shard (pre-pad)
        assert self.ns * n_cores == n_nodes
        self.t = (self.ns + P - 1) // P        # dst tiles per core
        assert self.t % 2 == 0
        self.ta = self.t // 2                  # tiles in table half A
        self.nsp = self.t * P                  # padded shard size
        self.ntot = self.nsp * n_cores         # padded global table rows
        self.na = self.ntot // 2               # rows in table half A
        self.n_chunks = n_chunks               # int16 table chunks
        assert self.ntot % n_chunks == 0
        self.vc = self.ntot // n_chunks        # rows per chunk
        assert self.vc < 32768
        self.mm_free = mm_free
        # set by preprocess:
        self.kb = None                         # [t][cb] blocks per bucket
        self.bt = None                         # [t] total blocks per tile
        self.btmax = None
        self.kbmax = None


# ----------------------------------------------------------------- host side

def preprocess(x, edge_index, W1, b1, W2, b2, cfg):
    """Shard + permute nodes, bucket edges by (dst tile, src chunk)."""
    N, C = cfg.n_nodes, cfg.n_cores
    src = np.asarray(edge_index[0], dtype=np.int64)
    dst = np.asarray(edge_index[1], dtype=np.int64)
    loop = np.arange(N, dtype=np.int64)
    src = np.concatenate([src, loop])
    dst = np.concatenate([dst, loop])

    deg = np.bincount(dst, minlength=N).astype(np.float64)
    dinv = (1.0 / np.sqrt(deg)).astype(np.float32)

    # deal nodes round-robin by in-degree across cores, then snake across
    # tiles within each core, to balance (core, tile, chunk) bucket counts.
    indeg = np.bincount(dst, minlength=N)
    order_glob = np.argsort(-indeg, kind="stable")
    shard_of = np.zeros(N, dtype=np.int64)
    node_tile = np.zeros(N, dtype=np.int64)
    node_col = np.zeros(N, dtype=np.int64)
    pg = np.zeros(N, dtype=np.int64)
    nodes_of = []
    snake = np.concatenate([np.arange(cfg.t), np.arange(cfg.t)[::-1]])
    tiles_seq = np.tile(snake, (P + 1) // 2 + 1)[: cfg.nsp]
    for c in range(C):
        order = order_glob[c::C]               # this core's nodes, by degree
        shard_of[order] = c
        tl = tiles_seq[: cfg.ns]
        node_tile[order] = tl
        pos = np.argsort(tl, kind="stable")
        cols = np.empty(cfg.ns, dtype=np.int64)
        tile_sorted = tl[pos]
        start = np.searchsorted(tile_sorted, np.arange(cfg.t))
        cols[pos] = np.arange(cfg.ns) - start[tile_sorted]
        node_col[order] = cols
        if SPLIT_AG:
            half_b = tl >= cfg.ta
            pg[order] = np.where(
                half_b,
                cfg.na + c * cfg.ta * P + (tl - cfg.ta) * P + cols,
                c * cfg.ta * P + tl * P + cols)
        else:
            pg[order] = c * cfg.nsp + tl * P + cols
        nv = np.full(cfg.nsp, -1, dtype=np.int64)
        nv[tl * P + cols] = order
        nodes_of.append(nv)

    # ---- bucket edges by (core, dst tile, src chunk)
    e_shard = shard_of[dst]
    e_tile = node_tile[dst]
    e_src_pg = pg[src]
    e_chunk = e_src_pg // cfg.vc
    e_dcol = node_col[dst]
    NB = cfg.n_chunks
    counts = np.zeros((C, cfg.t, NB), dtype=np.int64)
    np.add.at(counts, (e_shard, e_tile, e_chunk), 1)
    kb = ((counts.max(axis=0) + P - 1) // P).astype(np.int64)   # [t, NB]
    kb = np.maximum(kb, 1)
    # shared (max-over-cores) real index count per bucket; trailing slots up
    # to kb*128 are -1 and skipped by the gather ucode
    cfg.cmax = np.maximum(counts.max(axis=0), 16).astype(np.int64)
    # per-tile block totals must be even for DoubleRow pairing
    odd = kb.sum(axis=1) % 2 == 1
    kb[odd, NB - 1] += 1
    cfg.kb = kb
    cfg.bt = kb.sum(axis=1)                   # [t]
    cfg.btmax = int(cfg.bt.max())
    if cfg.btmax % 2:
        cfg.btmax += 1
    cfg.kbmax = int(kb.max())
    nblk_tot = int(cfg.bt.sum())

    order_all = np.lexsort((e_src_pg, e_chunk, e_tile, e_shard))
    s_sorted = (e_src_pg - e_chunk * cfg.vc)[order_all].astype(np.int16)
    d_sorted = e_dcol[order_all].astype(np.int64)
    key = (e_shard * cfg.t + e_tile)[order_all] * NB + e_chunk[order_all]
    bounds = np.searchsorted(key, np.arange(C * cfg.t * NB + 1))

    # block offsets per (t, cb)
    blk_off = np.zeros((cfg.t, NB), dtype=np.int64)
    run = 0
    for t in range(cfg.t):
        for cb in range(NB):
            blk_off[t, cb] = run
            run += kb[t, cb]

    xpad = np.zeros((cfg.kc * P, N), dtype=NPF8)
    xq = np.clip(np.asarray(x, dtype=np.float32), -240, 240)
    xpad[: cfg.f_in, :] = xq.T.astype(NPF8)
    w1h = np.zeros((P, cfg.kc, cfg.f_mid), dtype=NPF8)
    w1t = np.zeros((cfg.kc * P, cfg.f_mid), dtype=np.float32)
    w1t[: cfg.f_in] = np.clip(np.asarray(W1, dtype=np.float32) * WS, -240, 240)
    w1h[:] = w1t.reshape(cfg.kc, P, cfg.f_mid).transpose(1, 0, 2).astype(NPF8)
    w2h = np.zeros((P, cfg.kc2, cfg.f_out), dtype=np.float32)
    w2t = np.zeros((cfg.kc2 * P, cfg.f_out), dtype=np.float32)
    w2t[: cfg.f_mid] = np.asarray(W2, dtype=np.float32)
    w2h[:] = w2t.reshape(cfg.kc2, P, cfg.f_out).transpose(1, 0, 2)
    b1r = np.tile(np.asarray(b1, dtype=np.float32)[None, :], (P, 1))
    b2r = np.zeros((P, 8), dtype=np.float32)
    b2r[:, : cfg.f_out] = np.asarray(b2, dtype=np.float32)[None, :]

    cols128 = np.arange(P, dtype=np.int64)
    in_maps = []
    for c in range(C):
        nv = nodes_of[c]
        valid = nv >= 0
        xs = np.zeros((cfg.kc * P, cfg.nsp), dtype=NPF8)
        xs[:, valid] = xpad[:, nv[valid]]
        xt = np.ascontiguousarray(xs.reshape(cfg.kc, P, cfg.nsp).transpose(1, 0, 2))
        dvt = np.zeros(cfg.nsp, dtype=np.float32)
        dvt[valid] = dinv[nv[valid]]
        dv = np.ascontiguousarray(dvt.reshape(cfg.t, P).T)
        # idx: per (t, cb): kb*128 int16, idx j at [j%16, off*8 + j//16]
        eidx = np.zeros((P, nblk_tot * 8), dtype=np.int16)
        # S: per block b, S[p, b, col] = 1 if edge slot (b*128+p) -> dst col
        sful = np.zeros((P, nblk_tot, P), dtype=NPF8)
        for t in range(cfg.t):
            for cb in range(NB):
                lo = bounds[(c * cfg.t + t) * NB + cb]
                hi = bounds[(c * cfg.t + t) * NB + cb + 1]
                cnt = hi - lo
                nsl = int(kb[t, cb]) * P
                off = int(blk_off[t, cb])
                # trailing -1 idxs are skipped by the gather ucode; keep at
                # least 16 non-negative so every SDMA engine gets a desc
                cmv = int(cfg.cmax[t, cb])
                ai = np.full(nsl, -1 if DYN_CNT else 0, dtype=np.int16)
                if DYN_CNT:
                    ai[:cmv] = 0
                ai[:cnt] = s_sorted[lo:hi]
                eidx[:, off * 8: off * 8 + nsl // 16] = np.tile(
                    ai.reshape(nsl // 16, 16).T, (8, 1))
                ad = np.full(nsl, -1, dtype=np.int64)
                ad[:cnt] = d_sorted[lo:hi]
                blkd = ad.reshape(int(kb[t, cb]), P)      # [kb, 128] dst cols
                sful[:, off: off + int(kb[t, cb]), :] = (
                    blkd.T[:, :, None] == cols128[None, None, :]).astype(NPF8)
        in_maps.append({
            "xt": xt, "w1": w1h, "w2": w2h, "b1r": b1r, "b2r": b2r,
            "dinv_1": dv * (GS / WS), "dinv_3": dv / GS, "dinv_16": dv * GS,
            "eidx": eidx, "sful": sful,
        })
    return in_maps, nodes_of


# --------------------------------------------------------------- device side

def build(cfg, debug=False):
    nc = bacc.Bacc("TRN2", target_bir_lowering=False, debug=debug,
                   enable_asserts=False, num_devices=cfg.n_cores,
                   num_swdge_queues=4)
    T, NB = cfg.t, cfg.n_chunks
    FM, FMP, FO, KC, KC2 = cfg.f_mid, cfg.fmp, cfg.f_out, cfg.kc, cfg.kc2
    kb, bt, btmax, kbmax = cfg.kb, cfg.bt, cfg.btmax, cfg.kbmax
    nblk_tot = int(bt.sum())
    blk_off = np.zeros((T, NB), dtype=np.int64)
    run = 0
    for t in range(T):
        for cb in range(NB):
            blk_off[t, cb] = run
            run += kb[t, cb]
    tile_off = [int(blk_off[t, 0]) for t in range(T)]

    xt_d = nc.dram_tensor("xt", [P, KC, cfg.nsp], FP8, kind="ExternalInput").ap()
    w1_d = nc.dram_tensor("w1", [P, KC, FM], FP8, kind="ExternalInput").ap()
    w2_d = nc.dram_tensor("w2", [P, KC2, FO], F32, kind="ExternalInput").ap()
    b1_d = nc.dram_tensor("b1r", [P, FM], F32, kind="ExternalInput").ap()
    b2_d = nc.dram_tensor("b2r", [P, 8], F32, kind="ExternalInput").ap()
    dv1_d = nc.dram_tensor("dinv_1", [P, T], F32, kind="ExternalInput").ap()
    dv3_d = nc.dram_tensor("dinv_3", [P, T], F32, kind="ExternalInput").ap()
    dv16_d = nc.dram_tensor("dinv_16", [P, T], F32, kind="ExternalInput").ap()
    ei_d = nc.dram_tensor("eidx", [P, nblk_tot * 8], I16, kind="ExternalInput").ap()
    sf_d = nc.dram_tensor("sful", [P, nblk_tot, P], FP8, kind="ExternalInput").ap()
    out_d = nc.dram_tensor("out", [cfg.nsp, FO], F32, kind="ExternalOutput").ap()

    rg = [list(range(cfg.n_cores))]

    with tile.TileContext(nc) as tc:
        with tc.tile_pool(name="res", bufs=1) as res, \
             tc.tile_pool(name="dram", bufs=1, space="DRAM") as dram:
            g_local = dram.tile([cfg.nsp, FMP], FP8)
            g2_local = dram.tile([cfg.nsp, 256], FP8)
            if SPLIT_AG:
                g_full_a = dram.tile([cfg.na, FMP], FP8, addr_space="Shared")
                g_full_b = dram.tile([cfg.na, FMP], FP8, addr_space="Shared")
                g2_full_a = dram.tile([cfg.na, 256], FP8, addr_space="Shared")
                g2_full_b = dram.tile([cfg.na, 256], FP8, addr_space="Shared")
            else:
                g_full_a = dram.tile([cfg.ntot, FMP], FP8, addr_space="Shared")
                g_full_b = g_full_a
                g2_full_a = dram.tile([cfg.ntot, 256], FP8, addr_space="Shared")
                g2_full_b = g2_full_a

            w2_sb = res.tile([P, KC2, FO], F32)
            nc.sync.dma_start(out=w2_sb[:], in_=w2_d[:])
            b1_sb = res.tile([P, FM], F32)
            nc.sync.dma_start(out=b1_sb[:], in_=b1_d[:])
            b2_sb = res.tile([P, 8], F32)
            nc.sync.dma_start(out=b2_sb[:], in_=b2_d[:])
            dv1_sb = res.tile([P, T], F32)
            nc.sync.dma_start(out=dv1_sb[:], in_=dv1_d[:])
            dv3_sb = res.tile([P, T], F32)
            nc.sync.dma_start(out=dv3_sb[:], in_=dv3_d[:])
            dv16_sb = res.tile([P, T], F32)
            nc.sync.dma_start(out=dv16_sb[:], in_=dv16_d[:])
            ident = res.tile([P, P], F32)
            make_identity(nc, ident[:])

            # ---------------- phase 1: g = fp8(GS * dinv * (x @ W1))
            with tc.tile_pool(name="p1", bufs=3) as p1, \
                 tc.tile_pool(name="p1w", bufs=1) as p1w, \
                 tc.tile_pool(name="p1ps", bufs=2, space="PSUM") as p1ps:
                w1_sb = p1w.tile([P, KC, FM], FP8)
                nc.sync.dma_start(out=w1_sb[:], in_=w1_d[:])
                for t in range(T):
                    xtile = p1.tile([P, KC, P], FP8, tag="xtile")
                    nc.sync.dma_start(out=xtile[:], in_=xt_d[:, :, t * P:(t + 1) * P])
                    hp = p1ps.tile([P, FM], F32, tag="hp")
                    for f0 in range(0, FM, cfg.mm_free):
                        f1 = min(f0 + cfg.mm_free, FM)
                        for c in range(0, KC, 2):
                            nc.tensor.matmul(
                                out=hp[:, f0:f1], lhsT=xtile[:, c:c + 2, :],
                                rhs=w1_sb[:, c:c + 2, f0:f1],
                                start=(c == 0), stop=(c == KC - 2),
                                perf_mode=DR)
                    gt = p1.tile([P, FMP], FP8, tag="gt")
                    if t < 3:
                        nc.vector.memset(gt[:, FM:], 0.0)
                    nc.vector.tensor_scalar(
                        out=gt[:, :FM], in0=hp[:], scalar1=dv1_sb[:, t:t + 1],
                        scalar2=None, op0=ALU.mult)
                    nc.sync.dma_start(out=g_local[t * P:(t + 1) * P, :], in_=gt[:])

            # ---------------- phase 2: allgather g (two halves for overlap)
            if SPLIT_AG:
                nc.gpsimd.collective_compute(
                    "AllGather", ALU.bypass, replica_groups=rg,
                    ins=[g_local[0:cfg.ta * P, :]], outs=[g_full_a[:]])
                nc.gpsimd.collective_compute(
                    "AllGather", ALU.bypass, replica_groups=rg,
                    ins=[g_local[cfg.ta * P:, :]], outs=[g_full_b[:]])
            else:
                nc.gpsimd.collective_compute(
                    "AllGather", ALU.bypass, replica_groups=rg,
                    ins=[g_local[:]], outs=[g_full_a[:]])

            # ---------------- phase 3
            with tc.tile_pool(name="p3", bufs=3) as p3, \
                 tc.tile_pool(name="p3g", bufs=3) as p3g, \
                 tc.tile_pool(name="p3acc", bufs=2, space="PSUM") as p3acc, \
                 tc.tile_pool(name="p3ps", bufs=2, space="PSUM") as p3ps:
                nfs = (FM + cfg.mm_free - 1) // cfg.mm_free
                for t in range(T):
                    btt = int(bt[t])
                    o_t = tile_off[t]
                    eit = p3.tile([P, btmax * 8], I16, tag="eit")
                    nc.sync.dma_start(
                        out=eit[:, : btt * 8],
                        in_=ei_d[:, o_t * 8: (o_t + btt) * 8])
                    sst = p3.tile([P, btmax, P], FP8, tag="sst")
                    nc.sync.dma_start(
                        out=sst[:, :btt, :], in_=sf_d[:, o_t: o_t + btt, :])
                    gg = p3g.tile([P, btmax, FMP], FP8, tag="gg")
                    if t < 3:
                        nc.vector.memset(gg[:, :, :], 0.0)
                    for cb in range(NB):
                        kbb = int(kb[t, cb])
                        ni = kbb * P
                        co = int(blk_off[t, cb]) - o_t
                        rv = int(cfg.cmax[t, cb]) if DYN_CNT else ni
                        if SPLIT_AG:
                            ghalf = g_full_a if cb < NB // 2 else g_full_b
                            coff = (cb % (NB // 2)) * cfg.vc
                        else:
                            ghalf, coff = g_full_a, cb * cfg.vc
                        nc.gpsimd.dma_gather(
                            out_ap=gg[:, co:co + kbb, :],
                            in_ap=ghalf[coff:coff + cfg.vc, :],
                            idxs_ap=eit[:, co * 8: co * 8 + ni // 16],
                            num_idxs=ni, num_idxs_reg=rv, elem_size=FMP,
                            single_packet=(ni <= 1024), queue_num=(t * NB + cb) % 4)
                    acc = p3acc.tile([P, FM], F32, tag="acc")
                    for b in range(0, btt, 2):
                        for fi in range(nfs):
                            f0 = fi * cfg.mm_free
                            f1 = min(f0 + cfg.mm_free, FM)
                            nc.tensor.matmul(
                                out=acc[:, f0:f1], lhsT=sst[:, b:b + 2, :],
                                rhs=gg[:, b:b + 2, f0:f1],
                                start=(b == 0), stop=(b == btt - 2),
                                perf_mode=DR)
                    # epilogue: out1 = relu(dinv/GS*acc + b1)
                    t1 = p3.tile([P, FM], F32, tag="t1")
                    nc.vector.tensor_scalar(
                        out=t1[:], in0=acc[:], scalar1=dv3_sb[:, t:t + 1],
                        scalar2=None, op0=ALU.mult)
                    nc.vector.tensor_add(out=t1[:], in0=t1[:], in1=b1_sb[:])
                    nc.vector.tensor_scalar_max(out=t1[:], in0=t1[:], scalar1=0.0)
                    # g2T = W2^T @ t1^T
                    g2t = p3ps.tile([P, P], F32, tag="g2t")
                    for c in range(KC2):
                        f0 = c * P
                        cw = min(P, FM - f0)
                        tp = p3ps.tile([P, P], F32, tag="tp")
                        nc.tensor.transpose(
                            out=tp[:cw, :], in_=t1[:, f0:f0 + cw], identity=ident[:])
                        tps = p3.tile([P, P], F32, tag="tps")
                        nc.vector.tensor_copy(out=tps[:cw, :], in_=tp[:cw, :])
                        nc.tensor.matmul(
                            out=g2t[:FO, :], lhsT=w2_sb[:cw, c, :], rhs=tps[:cw, :],
                            start=(c == 0), stop=(c == KC2 - 1))
                    drp = p3ps.tile([P, P], F32, tag="tp")
                    nc.tensor.transpose(
                        out=drp[:], in_=dv16_sb[:, t:t + 1].to_broadcast([P, P]),
                        identity=ident[:])
                    dr = p3.tile([P, P], F32, tag="dr")
                    nc.vector.tensor_copy(out=dr[:], in_=drp[:])
                    g2s = p3.tile([P, P], F32, tag="g2s")
                    nc.vector.tensor_tensor(
                        out=g2s[:FO, :], in0=g2t[:FO, :], in1=dr[:FO, :], op=ALU.mult)
                    g2ntp = p3ps.tile([P, 8], F32, tag="tp")
                    nc.tensor.transpose(
                        out=g2ntp[:, :FO], in_=g2s[:FO, :], identity=ident[:FO, :FO])
                    g2o = p3.tile([P, 256], FP8, tag="g2o")
                    nc.vector.memset(g2o[:], 0.0)
                    nc.vector.tensor_copy(out=g2o[:, :FO], in_=g2ntp[:, :FO])
                    nc.sync.dma_start(
                        out=g2_local[t * P:(t + 1) * P, :], in_=g2o[:])

            # ---------------- phase 3.5: allgather g2 (two halves)
            if SPLIT_AG:
                nc.gpsimd.collective_compute(
                    "AllGather", ALU.bypass, replica_groups=rg,
                    ins=[g2_local[0:cfg.ta * P, :]], outs=[g2_full_a[:]])
                nc.gpsimd.collective_compute(
                    "AllGather", ALU.bypass, replica_groups=rg,
                    ins=[g2_local[cfg.ta * P:, :]], outs=[g2_full_b[:]])
            else:
                nc.gpsimd.collective_compute(
                    "AllGather", ALU.bypass, replica_groups=rg,
                    ins=[g2_local[:]], outs=[g2_full_a[:]])

            # ---------------- phase 4
            with tc.tile_pool(name="p4", bufs=3) as p4, \
                 tc.tile_pool(name="p4g", bufs=3) as p4g, \
                 tc.tile_pool(name="p4ps", bufs=2, space="PSUM") as p4ps:
                for t in range(T):
                    btt = int(bt[t])
                    o_t = tile_off[t]
                    eit = p4.tile([P, btmax * 8], I16, tag="eit4")
                    nc.sync.dma_start(
                        out=eit[:, : btt * 8],
                        in_=ei_d[:, o_t * 8: (o_t + btt) * 8])
                    sst = p4.tile([P, btmax, P], FP8, tag="sst4")
                    nc.sync.dma_start(
                        out=sst[:, :btt, :], in_=sf_d[:, o_t: o_t + btt, :])
                    gg2 = p4g.tile([P, btmax, 256], FP8, tag="gg2")
                    if t < 3:
                        nc.vector.memset(gg2[:, :, :], 0.0)
                    for cb in range(NB):
                        kbb = int(kb[t, cb])
                        ni = kbb * P
                        co = int(blk_off[t, cb]) - o_t
                        rv = int(cfg.cmax[t, cb]) if DYN_CNT else ni
                        if SPLIT_AG:
                            ghalf = g2_full_a if cb < NB // 2 else g2_full_b
                            coff = (cb % (NB // 2)) * cfg.vc
                        else:
                            ghalf, coff = g2_full_a, cb * cfg.vc
                        nc.gpsimd.dma_gather(
                            out_ap=gg2[:, co:co + kbb, :],
                            in_ap=ghalf[coff:coff + cfg.vc, :],
                            idxs_ap=eit[:, co * 8: co * 8 + ni // 16],
                            num_idxs=ni, num_idxs_reg=rv, elem_size=256,
                            single_packet=(ni <= 1024), queue_num=(t * NB + cb) % 4)
                    acc2 = p4ps.tile([P, P], F32, tag="acc2")
                    for b in range(0, btt, 2):
                        nc.tensor.matmul(
                            out=acc2[:8, :], lhsT=gg2[:, b:b + 2, :8],
                            rhs=sst[:, b:b + 2, :],
                            start=(b == 0), stop=(b == btt - 2),
                            perf_mode=DR)
                    t2s = p4.tile([P, P], F32, tag="t2s")
                    nc.vector.tensor_copy(out=t2s[:8, :], in_=acc2[:8, :])
                    t2ntp = p4ps.tile([P, 8], F32, tag="t2ntp")
                    nc.tensor.transpose(
                        out=t2ntp[:, :8], in_=t2s[:8, :], identity=ident[:8, :8])
                    tf = p4.tile([P, 8], F32, tag="tf")
                    nc.vector.tensor_scalar(
                        out=tf[:], in0=t2ntp[:], scalar1=dv3_sb[:, t:t + 1],
                        scalar2=None, op0=ALU.mult)
                    nc.vector.tensor_add(out=tf[:], in0=tf[:], in1=b2_sb[:])
                    nm = p4.tile([P, 1], F32, tag="nm")
                    nc.vector.tensor_reduce(
                        out=nm[:], in_=tf[:, :FO], axis=AX.X, op=ALU.max, negate=True)
                    ex = p4.tile([P, 8], F32, tag="ex")
                    se = p4.tile([P, 1], F32, tag="se")
                    nc.scalar.activation(
                        out=ex[:, :FO], in_=tf[:, :FO], func=ACT.Exp,
                        bias=nm[:, :1], scale=1.0, accum_out=se[:, :1])
                    lse = p4.tile([P, 1], F32, tag="lse")
                    nc.scalar.activation(out=lse[:], in_=se[:], func=ACT.Ln)
                    of = p4.tile([P, 8], F32, tag="of")
                    nc.vector.tensor_scalar(
                        out=of[:, :FO], in0=tf[:, :FO], scalar1=nm[:, :1],
                        scalar2=lse[:, :1], op0=ALU.add, op1=ALU.subtract)
                    nc.sync.dma_start(out=out_d[t * P:(t + 1) * P, :], in_=of[:, :FO])

    nc.compile()
    return nc


# ------------------------------------------------------------------ runner

def _run(inputs, cfg=None, trace=False, trace_kwargs=None):
    cfg = cfg or Cfg()
    in_maps, nodes_of = preprocess(
        inputs["x"], inputs["edge_index"], inputs["W1"], inputs["b1"],
        inputs["W2"], inputs["b2"], cfg)
    nc = build(cfg)
    res = bass_utils.run_bass_kernel_spmd(
        nc, in_maps, core_ids=list(range(cfg.n_cores)), trace=trace,
        **(trace_kwargs or {}))
    out = np.zeros((cfg.n_nodes, cfg.f_out), dtype=np.float32)
    for c in range(cfg.n_cores):
        oc = res.results[c]["out"]
        nv = nodes_of[c]
        valid = nv >= 0
        out[nv[valid]] = oc[valid]
    return out, res


def kernel(**inputs):
    out, _ = _run(inputs)
    return out
